# revision 15
# baseline (speedup 1.0000x reference)
"""DANet-style dual attention (PAM + CAM) distributed Bass kernel for 8 TRN2 NeuronCores.

Problem shapes (hardcoded): x (4, 512, 64, 64) fp32, Wq/Wk (64, 512), Wv (512, 512),
biases, scalar gammas.  N = 64*64 = 4096 positions per batch image.

Sharding: data-parallel over (batch, N-half): core c handles batch c//2 and query
rows [h*2048, (h+1)*2048) with h = c%2.  Each core holds the full (rotated) image
for K/V, computes its half of the position attention, then the pair of cores
all-reduces the 512x512 channel Gram matrix (cen) for the CAM stage.

All 8 cores run an IDENTICAL program: the host rotates each core's input columns
so that "my" query slice is always columns [0:2048].  Softmax/attention sums are
permutation-invariant, so the rotation only permutes the j-axis.

Precision: bf16 TensorEngine operands (full-rate) everywhere except the channel
Gram matrix (cen = pam pam^T), which is computed from fp32 pam in plain fp32
(4 cyc/row) because its softmax logits span +-4096 and bf16 error there would
visibly distort the channel attention.  The residual path (x, pam) stays fp32.

Computation layout per core (C=512 channels, D=64 qk channels, NL=2048 local):
  - k  [64, 4096] bf16 (d on partitions)         k = Wk x + bk
  - q  [64, 2048] bf16 (d on partitions, local)  q = Wq x + bq
  - vT [4096, 512] bf16 (j on partitions)        v = Wv x  (bias via rank-1 trick)
  - energy^T tile [128 j, 512 i] = k_tile^T q_block   (PE)
  - st = exp(energy^T) (ACT, PSUM->SBUF, bf16)
  - rowsum[1, i]  += ones^T st                   (PE)
  - poutT[i, c]   += st_chunk^T vT_chunk         (PE)  + rowsum^T x bv rank-1
  - pamT[i, c] = poutT * (gamma_pam / rowsum[i]) + xT[i, c]  (one DVE op, fp32)
  - cen[c, d] += pamT_chunk^T pamT_chunk         (PE fp32) -> AllReduce over pair
  - pam[c, n] = PE-transpose(pamT) -> bf16       (hidden under the collective)
  - catt = softmax(rowmax(cen) - cen) == exp(rowmin(cen) - cen)/sum  (ACT+DVE)
  - cout[c, n] += cattT_chunk^T pam_chunk        (PE)
  - out = gamma_cam * cout + pam_fp32            (one DVE op) -> DMA out
"""

import functools

import numpy as np
import ml_dtypes

import concourse.bass as bass
import concourse.mybir as mybir
import concourse.tile as tile
from concourse import bacc
from concourse.bass import ts
from concourse.bass_utils import run_bass_kernel_spmd
from concourse.masks import make_identity

B, C, H, W = 4, 512, 64, 64
N = H * W          # 4096
D = C // 8         # 64
NCORES = 8
NL = N // 2        # 2048 local query positions per core

FP32 = mybir.dt.float32
BF16 = mybir.dt.bfloat16
NPBF16 = ml_dtypes.bfloat16


def _emit_iteration(tc, nc, pools, ext, consts, gamma_pam, gamma_cam, it, out_ap):
    """Emit one full dual-attention pipeline writing to out_ap ([C, NL] DRAM)."""
    (bigp, xbp, stp, xtip, outp, smallp, dramp, ps_acc, ps_str, ps_rs) = pools
    xb_e, xti_e = ext["xb"], ext["xti"]
    wqt_s, wkt_s, bq_s, bk_s, wvt_s, oh16, ones_bf, ident = consts
    Exp = mybir.ActivationFunctionType.Exp

    # ---------------- P1: projections ----------------
    qs = bigp.tile([D, NL], BF16, name=f"q_{it}", tag="q")
    ks = bigp.tile([D, N], BF16, name=f"k_{it}", tag="k")
    vts = bigp.tile([128, N // 128, C], BF16, name=f"vt_{it}", tag="vt")

    for t in range(N // 512):  # 8 tiles of 512 over full N
        xbt = xbp.tile([128, 4, 512], BF16, name=f"xbt_{it}_{t}", tag="xb")
        nc.sync.dma_start(
            xbt[:], xb_e[:, ts(t, 512)].rearrange("(k p) n -> p k n", p=128)
        )
        # k projection: psk[o, n] = sum_c Wk[o, c] x[c, n]
        psk = ps_acc.tile([D, 512], FP32, name=f"psk_{it}_{t}", tag="acc")
        for kk in range(4):
            nc.tensor.matmul(
                psk[:], wkt_s[:, kk, :], xbt[:, kk, :],
                start=(kk == 0), stop=(kk == 3),
            )
        nc.vector.tensor_scalar_add(ks[:, ts(t, 512)], psk[:], bk_s[:])
        if t < NL // 512:  # q only over local query columns
            psq = ps_acc.tile([D, 512], FP32, name=f"psq_{it}_{t}", tag="acc")
            for kk in range(4):
                nc.tensor.matmul(
                    psq[:], wqt_s[:, kk, :], xbt[:, kk, :],
                    start=(kk == 0), stop=(kk == 3),
                )
            nc.vector.tensor_scalar_add(qs[:, ts(t, 512)], psq[:], bq_s[:])
        # vT projection: psv[n, c] = sum_c' x[c', n] WvT[c', c]
        for s in range(4):
            psv = ps_str.tile([128, 512], FP32, name=f"psv_{it}_{t}_{s}", tag="s")
            for kk in range(4):
                nc.tensor.matmul(
                    psv[:], xbt[:, kk, ts(s, 128)], wvt_s[:, kk, :],
                    start=(kk == 0), stop=(kk == 3),
                )
            nc.vector.tensor_copy(vts[:, t * 4 + s, :], psv[:])

    # ---------------- P2: position attention ----------------
    pamts = bigp.tile([128, NL // 128, C], FP32, name=f"pamt_{it}", tag="pamt")
    for ib in range(NL // 512):  # 4 query blocks of 512
        rsp = ps_rs.tile([1, 512], FP32, name=f"rsp_{it}_{ib}", tag="rs")
        pops = [
            ps_acc.tile([128, 512], FP32, name=f"po_{it}_{ib}_{u}", tag="acc")
            for u in range(4)
        ]
        for j in range(N // 128):  # 32 key tiles of 128
            pse = ps_str.tile([128, 512], FP32, name=f"pse_{it}_{ib}_{j}", tag="s")
            nc.tensor.matmul(
                pse[:], ks[:, ts(j, 128)], qs[:, ts(ib, 512)],
                start=True, stop=True,
            )
            st = stp.tile([128, 512], BF16, name=f"st_{it}_{ib}_{j}", tag="st")
            nc.scalar.activation(st[:], pse[:], Exp)
            nc.tensor.matmul(
                rsp[:], ones_bf[:], st[:], start=(j == 0), stop=(j == N // 128 - 1)
            )
            for u in range(4):
                nc.tensor.matmul(
                    pops[u][:], st[:, ts(u, 128)], vts[:, j, :],
                    start=(j == 0), stop=False,
                )
        # transpose rowsum [1, 512] -> [128, 4] with 4 K=1 one-hot matmuls:
        # rsT[p, u] += rs[u*128+p] * onehot_u; bv bias is folded into xti on host.
        rs_b = smallp.tile([1, 512], BF16, name=f"rsb_{it}_{ib}", tag="rsb")
        nc.vector.tensor_copy(rs_b[:], rsp[:])
        rsT_ps = ps_rs.tile([128, 4], FP32, name=f"rstp_{it}_{ib}", tag="rst")
        for u in range(4):
            nc.tensor.matmul(
                rsT_ps[:], rs_b[:, ts(u, 128)], oh16[0:1, ts(u, 4)],
                start=(u == 0), stop=(u == 3),
            )
        recipT = smallp.tile([128, 4], FP32, name=f"rcp_{it}_{ib}", tag="rcp")
        nc.vector.reciprocal(recipT[:], rsT_ps[:])
        nc.vector.tensor_scalar_mul(recipT[:], recipT[:], float(gamma_pam))
        for u in range(4):
            t16 = ib * 4 + u
            xtt = xtip.tile([128, C], FP32, name=f"xtt_{it}_{t16}", tag="xti")
            nc.sync.dma_start(xtt[:], xti_e[ts(t16, 128), :])
            # pamT = pout_psum * (gamma_pam / rowsum) + xT   (single DVE op)
            nc.vector.scalar_tensor_tensor(
                out=pamts[:, t16, :], in0=pops[u][:], scalar=recipT[:, u : u + 1],
                in1=xtt[:], op0=mybir.AluOpType.mult, op1=mybir.AluOpType.add,
            )

    # ---------------- P3: channel Gram (cen) + AllReduce over the pair --------
    cen_sb = bigp.tile([128, 4, C], FP32, name=f"cen_{it}", tag="cen")
    for cc in range(4):
        psc = ps_acc.tile([128, 512], FP32, name=f"psc_{it}_{cc}", tag="acc")
        for nt in range(NL // 128):
            nc.tensor.matmul(
                psc[:], pamts[:, nt, ts(cc, 128)], pamts[:, nt, :],
                start=(nt == 0), stop=(nt == NL // 128 - 1),
            )
        nc.vector.tensor_copy(cen_sb[:, cc, :], psc[:])
    cen_in = dramp.tile([128, 4, C], FP32, name=f"cen_in_{it}", tag="cen_in")
    nc.sync.dma_start(cen_in[:], cen_sb[:])
    cen_out = dramp.tile([128, 4, C], FP32, name=f"cen_out_{it}", tag="cen_out")
    nc.gpsimd.collective_compute(
        "AllReduce",
        mybir.AluOpType.add,
        replica_groups=[[0, 1], [2, 3], [4, 5], [6, 7]],
        ins=[cen_in[:].opt()],
        outs=[cen_out[:].opt()],
    )
    cenf = bigp.tile([128, 4, C], FP32, name=f"cenf_{it}", tag="cenf")
    nc.sync.dma_start(cenf[:], cen_out[:])

    # ---------------- P4: transpose pamT -> pam  (overlaps the collective) ----
    # bf16 copy feeds the cout matmul; fp32 copy is the exact residual.
    pams = bigp.tile([128, 4, NL], BF16, name=f"pam_{it}", tag="pam")
    pamf = bigp.tile([128, 4, NL], FP32, name=f"pamf_{it}", tag="pamf")
    for t16 in range(NL // 128):
        for cc in range(4):
            pst = ps_str.tile([128, 128], FP32, name=f"pt_{it}_{t16}_{cc}", tag="s")
            nc.tensor.transpose(pst[:], pamts[:, t16, ts(cc, 128)], ident[:])
            nc.vector.tensor_copy(pams[:, cc, ts(t16, 128)], pst[:])
            nc.vector.tensor_copy(pamf[:, cc, ts(t16, 128)], pst[:])

    # ---------------- P5: channel attention weights ----------------
    cattts = bigp.tile([128, 4, C], BF16, name=f"cattt_{it}", tag="cattt")
    for cc in range(4):
        rmin = smallp.tile([128, 1], FP32, name=f"rmin_{it}_{cc}", tag="rmin")
        nc.vector.tensor_reduce(
            rmin[:], cenf[:, cc, :], axis=mybir.AxisListType.X, op=mybir.AluOpType.min
        )
        csum = smallp.tile([128, 1], FP32, name=f"csum_{it}_{cc}", tag="csum")
        # catt_unnorm = exp(rowmin - cen); jax softmax(max-cen) == exp(min-cen)/sum
        nc.scalar.activation(
            cenf[:, cc, :], cenf[:, cc, :], Exp,
            bias=rmin[:], scale=-1.0, accum_out=csum[:],
        )
        crec = smallp.tile([128, 1], FP32, name=f"crec_{it}_{cc}", tag="crec")
        nc.vector.reciprocal(crec[:], csum[:])
        nc.vector.tensor_scalar_mul(cenf[:, cc, :], cenf[:, cc, :], crec[:])
        for dd in range(4):
            pst = ps_str.tile([128, 128], FP32, name=f"ct_{it}_{cc}_{dd}", tag="s")
            nc.tensor.transpose(pst[:], cenf[:, cc, ts(dd, 128)], ident[:])
            nc.vector.tensor_copy(cattts[:, dd, ts(cc, 128)], pst[:])

    # ---------------- P6: channel attention output + residual ----------------
    for cc in range(4):
        for nt in range(NL // 512):
            pso = ps_acc.tile([128, 512], FP32, name=f"pso_{it}_{cc}_{nt}", tag="acc")
            for dd in range(4):
                nc.tensor.matmul(
                    pso[:], cattts[:, dd, ts(cc, 128)], pams[:, dd, ts(nt, 512)],
                    start=(dd == 0), stop=(dd == 3),
                )
            # out = gamma_cam * cout + pam; use the fp32 pam via transpose of pamT?
            # pams is bf16; residual precision comes from re-reading pamT is not
            # possible layout-wise, so keep a separate fp32 transposed residual.
            ot = outp.tile([128, 512], FP32, name=f"ot_{it}_{cc}_{nt}", tag="out")
            nc.vector.scalar_tensor_tensor(
                out=ot[:], in0=pso[:], scalar=float(gamma_cam),
                in1=pamf[:, cc, ts(nt, 512)],
                op0=mybir.AluOpType.mult, op1=mybir.AluOpType.add,
            )
            nc.sync.dma_start(out_ap[ts(cc, 128), ts(nt, 512)], ot[:])


def build(gamma_pam: float, gamma_cam: float, amp: int = 1) -> bacc.Bacc:
    """Build + compile the SPMD graph.  amp>1 replicates the pipeline for timing."""
    nc = bacc.Bacc("TRN2", target_bir_lowering=False, debug=False, num_devices=NCORES)

    ext = {
        "xb": nc.dram_tensor("xb", [C, N], BF16, kind="ExternalInput").ap(),
        "xti": nc.dram_tensor("xti", [NL, C], FP32, kind="ExternalInput").ap(),
    }
    wqt_e = nc.dram_tensor("wqt", [C, D], BF16, kind="ExternalInput").ap()
    wkt_e = nc.dram_tensor("wkt", [C, D], BF16, kind="ExternalInput").ap()
    bq_e = nc.dram_tensor("bq", [D, 1], FP32, kind="ExternalInput").ap()
    bk_e = nc.dram_tensor("bk", [D, 1], FP32, kind="ExternalInput").ap()
    wvt_e = nc.dram_tensor("wvt", [C, C], BF16, kind="ExternalInput").ap()
    out_e = nc.dram_tensor("out", [C, NL], FP32, kind="ExternalOutput").ap()
    scratch = (
        nc.dram_tensor("oscratch", [C, NL], FP32).ap() if amp > 1 else None
    )

    with tile.TileContext(nc) as tc:
        with (
            tc.tile_pool(name="const", bufs=1) as constp,
            tc.tile_pool(name="big", bufs=1) as bigp,
            tc.tile_pool(name="xbp", bufs=2) as xbp,
            tc.tile_pool(name="stp", bufs=6) as stp,
            tc.tile_pool(name="xtip", bufs=4) as xtip,
            tc.tile_pool(name="outp", bufs=4) as outp,
            tc.tile_pool(name="smallp", bufs=4) as smallp,
            tc.tile_pool(name="dramp", bufs=2, space="DRAM") as dramp,
            tc.tile_pool(name="ps_acc", bufs=4, space="PSUM") as ps_acc,
            tc.tile_pool(name="ps_str", bufs=2, space="PSUM") as ps_str,
            tc.tile_pool(name="ps_rs", bufs=1, space="PSUM") as ps_rs,
        ):
            # constants
            wqt_s = constp.tile([128, 4, D], BF16, name="wqt_s")
            nc.sync.dma_start(wqt_s[:], wqt_e.rearrange("(k p) o -> p k o", p=128))
            wkt_s = constp.tile([128, 4, D], BF16, name="wkt_s")
            nc.sync.dma_start(wkt_s[:], wkt_e.rearrange("(k p) o -> p k o", p=128))
            bq_s = constp.tile([D, 1], FP32, name="bq_s")
            nc.sync.dma_start(bq_s[:], bq_e[:])
            bk_s = constp.tile([D, 1], FP32, name="bk_s")
            nc.sync.dma_start(bk_s[:], bk_e[:])
            wvt_s = constp.tile([128, 4, C], BF16, name="wvt_s")
            nc.sync.dma_start(wvt_s[:], wvt_e.rearrange("(k p) c -> p k c", p=128))
            oh16 = constp.tile([1, 16], BF16, name="oh16")
            nc.vector.memset(oh16[:], 0.0)
            for u in range(4):
                nc.vector.memset(oh16[0:1, u * 4 + u : u * 4 + u + 1], 1.0)
            ones_bf = constp.tile([128, 1], BF16, name="ones_bf")
            nc.vector.memset(ones_bf[:], 1.0)
            ident = constp.tile([128, 128], FP32, name="ident")
            make_identity(nc, ident[:])
            consts = (wqt_s, wkt_s, bq_s, bk_s, wvt_s, oh16, ones_bf, ident)

            pools = (bigp, xbp, stp, xtip, outp, smallp, dramp, ps_acc, ps_str, ps_rs)
            for it in range(amp):
                target = out_e if it == amp - 1 else scratch
                _emit_iteration(
                    tc, nc, pools, ext, consts, gamma_pam, gamma_cam, it, target
                )

    nc.compile()
    return nc


@functools.lru_cache(maxsize=2)
def _cached_build(gp: float, gc: float, amp: int) -> bacc.Bacc:
    return build(gp, gc, amp)


def make_in_maps(x, Wq, bq, Wk, bk, Wv, bv, gamma_pam=0.0):
    """Per-core input dicts.  Core c: batch c//2, query half c%2 (columns rotated
    so the local query slice is always [0:NL])."""
    xf = np.ascontiguousarray(np.asarray(x, np.float32).reshape(B, C, N))
    wqt = np.ascontiguousarray(np.asarray(Wq, np.float32).T).astype(NPBF16)
    wkt = np.ascontiguousarray(np.asarray(Wk, np.float32).T).astype(NPBF16)
    wvt = np.ascontiguousarray(np.asarray(Wv, np.float32).T).astype(NPBF16)
    bq_c = np.ascontiguousarray(np.asarray(bq, np.float32).reshape(D, 1))
    bk_c = np.ascontiguousarray(np.asarray(bk, np.float32).reshape(D, 1))
    in_maps = []
    for c in range(NCORES):
        b, h = c // 2, c % 2
        xb = xf[b]
        if h:
            xb = np.ascontiguousarray(np.concatenate([xb[:, NL:], xb[:, :NL]], axis=1))
        # residual carries the folded PAM value-bias: gamma_pam * bv
        xti = np.ascontiguousarray(xb[:, :NL].T) + gamma_pam * np.asarray(
            bv, np.float32
        ).reshape(1, C)
        in_maps.append(
            dict(
                xb=xb.astype(NPBF16), xti=xti.astype(np.float32),
                wqt=wqt, wkt=wkt, bq=bq_c, bk=bk_c, wvt=wvt,
            )
        )
    return in_maps


def assemble_output(results):
    out = np.empty((B, C, N), np.float32)
    for c in range(NCORES):
        b, h = c // 2, c % 2
        out[b][:, h * NL : (h + 1) * NL] = results[c]["out"]
    return out.reshape(B, C, H, W)


def kernel(**inputs) -> np.ndarray:
    gp = float(np.asarray(inputs["gamma_pam"]).reshape(-1)[0])
    gc = float(np.asarray(inputs["gamma_cam"]).reshape(-1)[0])
    nc = _cached_build(gp, gc, 1)
    in_maps = make_in_maps(
        inputs["x"], inputs["Wq"], inputs["bq"], inputs["Wk"], inputs["bk"],
        inputs["Wv"], inputs["bv"], gamma_pam=gp,
    )
    res = run_bass_kernel_spmd(nc, in_maps, list(range(NCORES)))
    return assemble_output(res.results)
